# revision 19
# baseline (speedup 1.0000x reference)
"""Squared Euclidean distance matrix kernel for Trainium2 (Bass/Tile).

out[i, j] = ||mat_1[i]||^2 + ||mat_2[j]||^2 - 2 * mat_1[i] . mat_2[j]

Sharding: mat_1 rows (and output rows) split across 8 NeuronCores; mat_2
replicated.  Each core computes a (2048, 8192) tile independently.

v5 design (all-fp8-DoubleRow PE stream + fp16 output), based on HW
microbenchmarks:
  - at8 = fp8(A^T) [128, 2, 2048], bt8 = fp8(-2 B^T) [128, 2, 8192]
    (PE f32 transposes 4-at-a-time into one PSUM bank, 512-wide DVE
    scale-casts out).
  - Main tile (m, s): ONE DoubleRow fp8 matmul contracts all 256 dims
    (measured 216 ns), plus ONE zero-padded DoubleRow correction matmul
    (full K=128 partitions, only rows 0-1 nonzero; small-K matmuls and
    mixed-mode groups stall the PE stream, zero-padded same-shape DR
    does not):
      corr_l[0,c,m] = fp8(sq_a[m]/4) (hi, lo residual), corr_l[1,c,m]=4
      corr_r[0,c,n] = 4,  corr_r[1,c,n] = fp8(sq_b[n]/4) (hi, lo)
    so the pair contributes sq_a[m] + sq_b[n] exactly up to the fp8 lo
    residual (~0.5 abs).  sq_* are computed from the QUANTIZED tiles
    (squares on GPSIMD, fp8 0.25/1.0-column DoubleRow reduce on PE).
  - PSUM: two 1536-col (3-bank) main tiles double-buffered + 1 transpose
    bank + 1 row bank.  Evacuation in 1536/1024-wide chunks, split
    between DVE tensor_copy and ACT Copy, into fp16 staging; 1 MB DMA
    per (m, half).  Host upcasts fp16 -> f32.
"""

import numpy as np

import concourse.bass as bass
import concourse.mybir as mybir
from concourse import bacc
from contextlib import ExitStack
from concourse.tile import TileContext
from concourse.masks import make_identity

F32 = mybir.dt.float32
F32R = mybir.dt.float32r
F16 = mybir.dt.float16
FP8 = mybir.dt.float8e4
AX = mybir.AxisListType
OP = mybir.AluOpType
AF = mybir.ActivationFunctionType
DR = mybir.MatmulPerfMode.DoubleRow

N_CORES = 8
M_FULL, N_FULL, D_FULL = 16384, 8192, 256


def build(m_sh=M_FULL // N_CORES, n=N_FULL, d=D_FULL):
    P = 128
    FD = 512                      # psum bank width (f32)
    KC = d // P                   # 2 -> DoubleRow depth
    MT = m_sh // P                # m tiles per core (16)
    NB = n // P                   # b row tiles (64)
    AB = 4                        # row tiles per load batch
    out_w = 4096                  # out staging width == half width
    OH = n // out_w               # halves (2)
    GW = 1024                     # psum group width (2 banks)

    nc = bacc.Bacc()
    a = nc.dram_tensor("a", [m_sh, d], F32, kind="ExternalInput")
    b = nc.dram_tensor("b", [n, d], F32, kind="ExternalInput")
    o = nc.dram_tensor("out", [m_sh, n], F16, kind="ExternalOutput")

    with ExitStack() as ctx:
        tc = ctx.enter_context(TileContext(nc))
        singles = ctx.enter_context(tc.tile_pool(name="singles", bufs=1))
        persist = ctx.enter_context(tc.tile_pool(name="persist", bufs=1))
        natp = ctx.enter_context(tc.tile_pool(name="natp", bufs=3))
        outp = ctx.enter_context(tc.tile_pool(name="outp", bufs=3))
        psump = ctx.enter_context(tc.tile_pool(name="psump", bufs=1, space="PSUM"))

        identity = singles.tile([P, P], F32, tag="identity", name="identity")
        make_identity(nc, identity)

        # fp8 DR reduce columns [128, 2, 1]:
        #   A: sq_a/4 = 0.25 * sum(a^2)            -> 0.25
        #   B: sq_b/4 = (1/16) * sum((-2b)^2)      -> 0.0625
        # (DR weights need pair-stride %16 == 0 -> 16 columns wide, all
        # equal; the reduce result lands in psum rows 0..15, row 0 used)
        qcol8 = singles.tile([P, KC, 16], FP8, tag="qcol8", name="qcol8")
        nc.vector.memset(qcol8, 0.25)
        q16col8 = singles.tile([P, KC, 16], FP8, tag="q16col8", name="q16col8")
        nc.vector.memset(q16col8, 0.0625)

        # fp8 DR operand tiles
        bt8 = persist.tile([P, KC, n], FP8, tag="bt8", name="bt8")
        at8 = persist.tile([P, KC, m_sh], FP8, tag="at8", name="at8")
        # zero-padded DR correction operands
        c8l = persist.tile([P, KC, m_sh], FP8, tag="c8l", name="c8l")
        c8r = persist.tile([P, KC, n], FP8, tag="c8r", name="c8r")
        nc.gpsimd.memset(c8l, 0.0)
        nc.gpsimd.memset(c8r, 0.0)
        # compute engines cannot address partition offset 1; stage the 4.0
        # rows at partition 0 and DMA into place
        fours = singles.tile([1, KC, n], FP8, tag="fours", name="fours")
        nc.vector.memset(fours, 4.0)
        nc.gpsimd.dma_start(out=c8l[1:2, :, :], in_=fours[0:1, :, :m_sh])
        nc.gpsimd.dma_start(out=c8r[0:1, :, :], in_=fours[0:1, :, :])

        # row staging (partition 0), DMA'd into c8l/c8r partition rows
        rowst = persist.tile([1, 2, max(m_sh, n)], FP8, tag="rowst",
                             name="rowst")
        t16r = persist.tile([1, max(m_sh, n)], F16, tag="t16r", name="t16r")

        def sq_rows(src8, width, scol, dst_rows, nslices):
            """matmul-reduce squares of src8 (DR) -> psum rows, then
            hi/lo fp8 split (values scaled by 1/4 via scol=s/4)."""
            for s in range(nslices):
                sl = slice(s * FD, (s + 1) * FD)
                sq8 = natp.tile([P, KC, FD], FP8, tag="sq8", bufs=2,
                                name="sq8")
                nc.scalar.activation(sq8[:, 0, :], src8[:, 0, sl], AF.Square)
                nc.scalar.activation(sq8[:, 1, :], src8[:, 1, sl], AF.Square)
                ps = psump.tile([16, FD], F32, tag="mm", bufs=4,
                                name="ps_sq")
                nc.tensor.matmul(ps, scol, sq8, start=True, stop=True,
                                 perf_mode=DR)
                # hi = fp8(ps) (ps already scaled by scol = s/4)
                nc.scalar.activation(rowst[0:1, 0, sl], ps[0:1, :], AF.Copy)
                # t16 = ps (f16), lo = t16 - hi
                nc.scalar.activation(t16r[0:1, sl], ps[0:1, :], AF.Copy)
                nc.vector.tensor_tensor(out=rowst[0:1, 1, sl],
                                        in0=t16r[0:1, sl],
                                        in1=rowst[0:1, 0, sl],
                                        op=OP.subtract)
            nc.gpsimd.dma_start(out=dst_rows, in_=rowst[0:1, :, :width])

        # ---- per half: B prep emitted as chunks; the main loop of the
        # PREVIOUS half interleaves the next half's prep chunks so the PE
        # never hits a prep bubble at the half boundary ----
        nbh = (NB // OH)          # b row tiles per half (32)

        def b_load(h):
            h_bats = []
            for i in range(nbh // AB):
                b_nat = natp.tile([P, AB, d], F32, tag="bnat",
                                  bufs=nbh // AB, name="b_nat")
                r0 = (h * nbh + i * AB) * P
                src = b[r0:r0 + AB * P, :].rearrange("(t p) d -> p t d", p=P)
                nc.sync.dma_start(out=b_nat, in_=src)
                h_bats.append(b_nat)
            return h_bats

        def b_prep_chunks(h, h_bats):
            """Yield prep work for half h in small chunks (loads already
            issued by b_load)."""
            for q2 in range(nbh // (2 * AB)):
                t0 = h * nbh + q2 * 2 * AB
                bats = h_bats[2 * q2:2 * q2 + 2]
                for k in range(KC):
                    pt = psump.tile([P, 2 * AB * P], F32, tag="mm", bufs=4,
                                    name="pt_b")
                    for jj in range(2 * AB):
                        nc.tensor.transpose(
                            pt[:, jj * P:(jj + 1) * P],
                            bats[jj // AB][:, jj % AB, k * P:(k + 1) * P],
                            identity,
                        )
                    nc.vector.tensor_scalar_mul(
                        bt8[:, k, t0 * P:(t0 + 2 * AB) * P], pt, -2.0
                    )
                yield
            # sqb rows for this half -> c8r[1, :, half] (one slice per chunk)
            for s in range(out_w // FD):
                sl = slice(s * FD, (s + 1) * FD)
                asl = slice(h * out_w + s * FD, h * out_w + (s + 1) * FD)
                sq8 = natp.tile([P, KC, FD], FP8, tag="sq8", bufs=2,
                                name="sq8")
                nc.scalar.activation(sq8[:, 0, :], bt8[:, 0, asl], AF.Square)
                nc.scalar.activation(sq8[:, 1, :], bt8[:, 1, asl], AF.Square)
                ps = psump.tile([16, FD], F32, tag="mm", bufs=4,
                                name="ps_sq")
                nc.tensor.matmul(ps, q16col8, sq8, start=True, stop=True,
                                 perf_mode=DR)
                nc.scalar.activation(rowst[0:1, 0, sl], ps[0:1, :], AF.Copy)
                nc.scalar.activation(t16r[0:1, sl], ps[0:1, :], AF.Copy)
                nc.vector.tensor_tensor(out=rowst[0:1, 1, sl],
                                        in0=t16r[0:1, sl],
                                        in1=rowst[0:1, 0, sl],
                                        op=OP.subtract)
                # per-slice row placement so early main tiles aren't gated
                # on the whole half's sq chain
                nc.gpsimd.dma_start(
                    out=c8r[1:2, :, asl], in_=rowst[0:1, :, sl]
                )
                yield

        # ---- A chain: ALL loads issued first (the DMA queue streams
        # while the PE transposes trail the data), then transpose
        # 8-at-a-time into a 1024-col psum gen, 1024-wide fp8 cast out ----
        a_bats = []
        for i in range(MT // AB):
            a_nat = natp.tile([P, AB, d], F32, tag="anat", bufs=MT // AB,
                              name="a_nat")
            r0 = i * AB * P
            src = a[r0:r0 + AB * P, :].rearrange("(t p) d -> p t d", p=P)
            nc.sync.dma_start(out=a_nat, in_=src)
            a_bats.append(a_nat)
        b0_bats = b_load(0)
        for t8 in range(MT // (2 * AB)):
            bats = a_bats[2 * t8:2 * t8 + 2]
            for k in range(KC):
                pt = psump.tile([P, 2 * AB * P], F32, tag="mm", bufs=4,
                                name="pt_a")
                for jj in range(2 * AB):
                    nc.tensor.transpose(
                        pt[:, jj * P:(jj + 1) * P],
                        bats[jj // AB][:, jj % AB, k * P:(k + 1) * P],
                        identity,
                    )
                nc.vector.tensor_copy(
                    at8[:, k, t8 * 2 * AB * P:(t8 + 1) * 2 * AB * P], pt
                )
        # sq_a rows -> c8l[0, :, :]  (use onecol8/4 = memset 0.25 too;
        # A is unscaled so scol must be 1/4 = qcol8)
        sq_rows(at8, m_sh, qcol8, c8l[0:1, :, :], m_sh // FD)

        def main_half(h, prep):
            """Main loop for half h, pulling prep chunks for half h+1."""
            for m in range(MT):
                if prep is not None and m >= 2:
                    for _ in range(2):
                        next(prep, None)
                msl = slice(m * P, (m + 1) * P)
                ostage = outp.tile([P, out_w], F16, tag="ostage",
                                   name="ostage")
                for g in range(out_w // GW):
                    off = g * GW
                    wide = psump.tile([P, GW], F32, tag="mm", bufs=4,
                                      name="ps_mm")
                    for si in range(GW // FD):
                        nsl = slice(h * out_w + off + si * FD,
                                    h * out_w + off + (si + 1) * FD)
                        dst = wide[:, si * FD:(si + 1) * FD]
                        nc.tensor.matmul(dst, at8[:, :, msl],
                                         bt8[:, :, nsl], start=True,
                                         stop=False, perf_mode=DR)
                        nc.tensor.matmul(dst, c8l[:, :, msl],
                                         c8r[:, :, nsl], start=False,
                                         stop=True, perf_mode=DR)
                    # split each evacuation across DVE and ACT so the
                    # combined rate beats the PE fill rate
                    nc.vector.tensor_copy(ostage[:, off:off + GW // 2],
                                          wide[:, :GW // 2])
                    nc.scalar.activation(ostage[:, off + GW // 2:off + GW],
                                         wide[:, GW // 2:], AF.Copy)
                nc.sync.dma_start(
                    out=o[msl, h * out_w:(h + 1) * out_w], in_=ostage
                )
            if prep is not None:
                for _ in prep:
                    pass

        # half 0 prep runs up front; half 1 prep interleaves with main(0)
        for _ in b_prep_chunks(0, b0_bats):
            pass

        def prep1():
            bats = b_load(1)
            yield
            yield from b_prep_chunks(1, bats)

        main_half(0, prep1())
        main_half(1, None)
    nc.finalize()
    return nc


_CACHE = {}


def _get_nc():
    if "nc" not in _CACHE:
        _CACHE["nc"] = build()
    return _CACHE["nc"]


def run(mat_1, mat_2, trace=False, **kw):
    from concourse.bass_utils import run_bass_kernel_spmd

    a = np.ascontiguousarray(np.asarray(mat_1, dtype=np.float32))
    b = np.ascontiguousarray(np.asarray(mat_2, dtype=np.float32))
    assert a.shape == (M_FULL, D_FULL) and b.shape == (N_FULL, D_FULL)
    m_sh = M_FULL // N_CORES
    nc = _get_nc()
    in_maps = [
        {"a": a[c * m_sh:(c + 1) * m_sh], "b": b} for c in range(N_CORES)
    ]
    res = run_bass_kernel_spmd(
        nc, in_maps, core_ids=list(range(N_CORES)), trace=trace, **kw
    )
    out = np.concatenate(
        [np.asarray(r["out"], dtype=np.float32) for r in res.results], axis=0
    )
    return out, res


def kernel(mat_1, mat_2):
    return run(mat_1, mat_2)[0]


# revision 20
# speedup vs baseline: 1.0381x; 1.0381x over previous
"""Squared Euclidean distance matrix kernel for Trainium2 (Bass/Tile).

out[i, j] = ||mat_1[i]||^2 + ||mat_2[j]||^2 - 2 * mat_1[i] . mat_2[j]

Sharding: mat_1 rows (and output rows) split across 8 NeuronCores; mat_2
replicated.  Each core computes a (2048, 8192) tile independently.

v5 design (all-fp8-DoubleRow PE stream + fp16 output), based on HW
microbenchmarks:
  - at8 = fp8(A^T) [128, 2, 2048], bt8 = fp8(-2 B^T) [128, 2, 8192]
    (PE f32 transposes 4-at-a-time into one PSUM bank, 512-wide DVE
    scale-casts out).
  - Main tile (m, s): ONE DoubleRow fp8 matmul contracts all 256 dims
    (measured 216 ns), plus ONE zero-padded DoubleRow correction matmul
    (full K=128 partitions, only rows 0-1 nonzero; small-K matmuls and
    mixed-mode groups stall the PE stream, zero-padded same-shape DR
    does not):
      corr_l[0,c,m] = fp8(sq_a[m]/4) (hi, lo residual), corr_l[1,c,m]=4
      corr_r[0,c,n] = 4,  corr_r[1,c,n] = fp8(sq_b[n]/4) (hi, lo)
    so the pair contributes sq_a[m] + sq_b[n] exactly up to the fp8 lo
    residual (~0.5 abs).  sq_* are computed from the QUANTIZED tiles
    (squares on GPSIMD, fp8 0.25/1.0-column DoubleRow reduce on PE).
  - PSUM: two 1536-col (3-bank) main tiles double-buffered + 1 transpose
    bank + 1 row bank.  Evacuation in 1536/1024-wide chunks, split
    between DVE tensor_copy and ACT Copy, into fp16 staging; 1 MB DMA
    per (m, half).  Host upcasts fp16 -> f32.
"""

import numpy as np

import concourse.bass as bass
import concourse.mybir as mybir
from concourse import bacc
from contextlib import ExitStack
from concourse.tile import TileContext
from concourse.masks import make_identity

F32 = mybir.dt.float32
F32R = mybir.dt.float32r
F16 = mybir.dt.float16
FP8 = mybir.dt.float8e4
AX = mybir.AxisListType
OP = mybir.AluOpType
AF = mybir.ActivationFunctionType
DR = mybir.MatmulPerfMode.DoubleRow

N_CORES = 8
M_FULL, N_FULL, D_FULL = 16384, 8192, 256


def build(m_sh=M_FULL // N_CORES, n=N_FULL, d=D_FULL):
    P = 128
    FD = 512                      # psum bank width (f32)
    KC = d // P                   # 2 -> DoubleRow depth
    MT = m_sh // P                # m tiles per core (16)
    NB = n // P                   # b row tiles (64)
    AB = 4                        # row tiles per load batch
    out_w = 4096                  # out staging width == half width
    OH = n // out_w               # halves (2)
    GW = 1024                     # psum group width (2 banks)

    nc = bacc.Bacc()
    a = nc.dram_tensor("a", [m_sh, d], F32, kind="ExternalInput")
    b = nc.dram_tensor("b", [n, d], F32, kind="ExternalInput")
    o = nc.dram_tensor("out", [m_sh, n], F16, kind="ExternalOutput")

    with ExitStack() as ctx:
        tc = ctx.enter_context(TileContext(nc))
        singles = ctx.enter_context(tc.tile_pool(name="singles", bufs=1))
        persist = ctx.enter_context(tc.tile_pool(name="persist", bufs=1))
        natp = ctx.enter_context(tc.tile_pool(name="natp", bufs=3))
        outp = ctx.enter_context(tc.tile_pool(name="outp", bufs=3))
        psump = ctx.enter_context(tc.tile_pool(name="psump", bufs=1, space="PSUM"))

        identity = singles.tile([P, P], F32, tag="identity", name="identity")
        make_identity(nc, identity)

        # fp8 DR reduce columns [128, 2, 1]:
        #   A: sq_a/4 = 0.25 * sum(a^2)            -> 0.25
        #   B: sq_b/4 = (1/16) * sum((-2b)^2)      -> 0.0625
        # (DR weights need pair-stride %16 == 0 -> 16 columns wide, all
        # equal; the reduce result lands in psum rows 0..15, row 0 used)
        qcol8 = singles.tile([P, KC, 16], FP8, tag="qcol8", name="qcol8")
        nc.vector.memset(qcol8, 0.25)
        q16col8 = singles.tile([P, KC, 16], FP8, tag="q16col8", name="q16col8")
        nc.vector.memset(q16col8, 0.0625)

        # fp8 DR operand tiles
        bt8 = persist.tile([P, KC, n], FP8, tag="bt8", name="bt8")
        at8 = persist.tile([P, KC, m_sh], FP8, tag="at8", name="at8")
        # zero-padded DR correction operands
        c8l = persist.tile([P, KC, m_sh], FP8, tag="c8l", name="c8l")
        c8r = persist.tile([P, KC, n], FP8, tag="c8r", name="c8r")
        nc.gpsimd.memset(c8l, 0.0)
        nc.gpsimd.memset(c8r, 0.0)
        # compute engines cannot address partition offset 1; stage the 4.0
        # rows at partition 0 and DMA into place
        fours = singles.tile([1, KC, n], FP8, tag="fours", name="fours")
        nc.vector.memset(fours, 4.0)
        nc.gpsimd.dma_start(out=c8l[1:2, :, :], in_=fours[0:1, :, :m_sh])
        nc.gpsimd.dma_start(out=c8r[0:1, :, :], in_=fours[0:1, :, :])

        # row staging (partition 0), DMA'd into c8l/c8r partition rows
        rowst = persist.tile([1, 2, max(m_sh, n)], FP8, tag="rowst",
                             name="rowst")
        t16r = persist.tile([1, max(m_sh, n)], F16, tag="t16r", name="t16r")

        def sq_rows(src8, width, scol, dst_rows, nslices):
            """matmul-reduce squares of src8 (DR) -> psum rows, then
            hi/lo fp8 split (values scaled by 1/4 via scol=s/4)."""
            for s in range(nslices):
                sl = slice(s * FD, (s + 1) * FD)
                sq8 = natp.tile([P, KC, FD], FP8, tag="sq8", bufs=2,
                                name="sq8")
                nc.scalar.activation(sq8[:, 0, :], src8[:, 0, sl], AF.Square)
                nc.scalar.activation(sq8[:, 1, :], src8[:, 1, sl], AF.Square)
                ps = psump.tile([16, FD], F32, tag="mm", bufs=4,
                                name="ps_sq")
                nc.tensor.matmul(ps, scol, sq8, start=True, stop=True,
                                 perf_mode=DR)
                # hi = fp8(ps) (ps already scaled by scol = s/4)
                nc.scalar.activation(rowst[0:1, 0, sl], ps[0:1, :], AF.Copy)
                # t16 = ps (f16), lo = t16 - hi
                nc.scalar.activation(t16r[0:1, sl], ps[0:1, :], AF.Copy)
                nc.vector.tensor_tensor(out=rowst[0:1, 1, sl],
                                        in0=t16r[0:1, sl],
                                        in1=rowst[0:1, 0, sl],
                                        op=OP.subtract)
            nc.gpsimd.dma_start(out=dst_rows, in_=rowst[0:1, :, :width])

        # ---- per half: B prep emitted as chunks; the main loop of the
        # PREVIOUS half interleaves the next half's prep chunks so the PE
        # never hits a prep bubble at the half boundary ----
        nbh = (NB // OH)          # b row tiles per half (32)

        def b_load(h):
            h_bats = []
            for i in range(nbh // AB):
                b_nat = natp.tile([P, AB, d], F32, tag="bnat",
                                  bufs=nbh // AB, name="b_nat")
                r0 = (h * nbh + i * AB) * P
                src = b[r0:r0 + AB * P, :].rearrange("(t p) d -> p t d", p=P)
                nc.sync.dma_start(out=b_nat, in_=src)
                h_bats.append(b_nat)
            return h_bats

        def b_prep_chunks(h, h_bats):
            """Yield prep work for half h in small chunks (loads already
            issued by b_load).  Squares run on ACT for half 0 (idle during
            the head) and on GPSIMD for half 1 (ACT is busy evacuating
            during the half-0 main loop)."""
            for q2 in range(nbh // (2 * AB)):
                t0 = h * nbh + q2 * 2 * AB
                bats = h_bats[2 * q2:2 * q2 + 2]
                for k in range(KC):
                    pt = psump.tile([P, 2 * AB * P], F32, tag="mm", bufs=4,
                                    name="pt_b")
                    for jj in range(2 * AB):
                        nc.tensor.transpose(
                            pt[:, jj * P:(jj + 1) * P],
                            bats[jj // AB][:, jj % AB, k * P:(k + 1) * P],
                            identity,
                        )
                    nc.vector.tensor_scalar_mul(
                        bt8[:, k, t0 * P:(t0 + 2 * AB) * P], pt, -2.0
                    )
                yield
            # sqb rows for this half -> c8r[1, :, half] (one slice per chunk)
            for s in range(out_w // FD):
                sl = slice(s * FD, (s + 1) * FD)
                asl = slice(h * out_w + s * FD, h * out_w + (s + 1) * FD)
                sq8 = natp.tile([P, KC, FD], FP8, tag="sq8", bufs=2,
                                name="sq8")
                if h == 0:
                    nc.scalar.activation(sq8[:, 0, :], bt8[:, 0, asl],
                                         AF.Square)
                    nc.scalar.activation(sq8[:, 1, :], bt8[:, 1, asl],
                                         AF.Square)
                else:
                    nc.gpsimd.tensor_mul(sq8[:, 0, :], bt8[:, 0, asl],
                                         bt8[:, 0, asl])
                    nc.gpsimd.tensor_mul(sq8[:, 1, :], bt8[:, 1, asl],
                                         bt8[:, 1, asl])
                ps = psump.tile([16, FD], F32, tag="mm", bufs=4,
                                name="ps_sq")
                nc.tensor.matmul(ps, q16col8, sq8, start=True, stop=True,
                                 perf_mode=DR)
                nc.scalar.activation(rowst[0:1, 0, sl], ps[0:1, :], AF.Copy)
                nc.scalar.activation(t16r[0:1, sl], ps[0:1, :], AF.Copy)
                nc.vector.tensor_tensor(out=rowst[0:1, 1, sl],
                                        in0=t16r[0:1, sl],
                                        in1=rowst[0:1, 0, sl],
                                        op=OP.subtract)
                # per-slice row placement so early main tiles aren't gated
                # on the whole half's sq chain
                nc.gpsimd.dma_start(
                    out=c8r[1:2, :, asl], in_=rowst[0:1, :, sl]
                )
                yield

        # ---- A chain: ALL loads issued first (the DMA queue streams
        # while the PE transposes trail the data), then transpose
        # 8-at-a-time into a 1024-col psum gen, 1024-wide fp8 cast out ----
        a_bats = []
        for i in range(MT // AB):
            a_nat = natp.tile([P, AB, d], F32, tag="anat", bufs=MT // AB,
                              name="a_nat")
            r0 = i * AB * P
            src = a[r0:r0 + AB * P, :].rearrange("(t p) d -> p t d", p=P)
            nc.sync.dma_start(out=a_nat, in_=src)
            a_bats.append(a_nat)
        b0_bats = b_load(0)
        for t8 in range(MT // (2 * AB)):
            bats = a_bats[2 * t8:2 * t8 + 2]
            for k in range(KC):
                pt = psump.tile([P, 2 * AB * P], F32, tag="mm", bufs=4,
                                name="pt_a")
                for jj in range(2 * AB):
                    nc.tensor.transpose(
                        pt[:, jj * P:(jj + 1) * P],
                        bats[jj // AB][:, jj % AB, k * P:(k + 1) * P],
                        identity,
                    )
                nc.vector.tensor_copy(
                    at8[:, k, t8 * 2 * AB * P:(t8 + 1) * 2 * AB * P], pt
                )
        # sq_a rows -> c8l[0, :, :]  (use onecol8/4 = memset 0.25 too;
        # A is unscaled so scol must be 1/4 = qcol8)
        sq_rows(at8, m_sh, qcol8, c8l[0:1, :, :], m_sh // FD)

        def main_half(h, prep):
            """Main loop for half h, pulling prep chunks for half h+1."""
            for m in range(MT):
                if prep is not None and m >= 2:
                    for _ in range(2):
                        next(prep, None)
                msl = slice(m * P, (m + 1) * P)
                ostage = outp.tile([P, out_w], F16, tag="ostage",
                                   name="ostage")
                for g in range(out_w // GW):
                    off = g * GW
                    wide = psump.tile([P, GW], F32, tag="mm", bufs=4,
                                      name="ps_mm")
                    for si in range(GW // FD):
                        nsl = slice(h * out_w + off + si * FD,
                                    h * out_w + off + (si + 1) * FD)
                        dst = wide[:, si * FD:(si + 1) * FD]
                        nc.tensor.matmul(dst, at8[:, :, msl],
                                         bt8[:, :, nsl], start=True,
                                         stop=False, perf_mode=DR)
                        nc.tensor.matmul(dst, c8l[:, :, msl],
                                         c8r[:, :, nsl], start=False,
                                         stop=True, perf_mode=DR)
                    # split each evacuation across DVE and ACT so the
                    # combined rate beats the PE fill rate
                    nc.vector.tensor_copy(ostage[:, off:off + GW // 2],
                                          wide[:, :GW // 2])
                    nc.scalar.activation(ostage[:, off + GW // 2:off + GW],
                                         wide[:, GW // 2:], AF.Copy)
                nc.sync.dma_start(
                    out=o[msl, h * out_w:(h + 1) * out_w], in_=ostage
                )
            if prep is not None:
                for _ in prep:
                    pass

        # half 0 prep runs up front; half 1 prep interleaves with main(0)
        for _ in b_prep_chunks(0, b0_bats):
            pass

        def prep1():
            bats = b_load(1)
            yield
            yield from b_prep_chunks(1, bats)

        main_half(0, prep1())
        main_half(1, None)
    nc.finalize()
    return nc


_CACHE = {}


def _get_nc():
    if "nc" not in _CACHE:
        _CACHE["nc"] = build()
    return _CACHE["nc"]


def run(mat_1, mat_2, trace=False, **kw):
    from concourse.bass_utils import run_bass_kernel_spmd

    a = np.ascontiguousarray(np.asarray(mat_1, dtype=np.float32))
    b = np.ascontiguousarray(np.asarray(mat_2, dtype=np.float32))
    assert a.shape == (M_FULL, D_FULL) and b.shape == (N_FULL, D_FULL)
    m_sh = M_FULL // N_CORES
    nc = _get_nc()
    in_maps = [
        {"a": a[c * m_sh:(c + 1) * m_sh], "b": b} for c in range(N_CORES)
    ]
    res = run_bass_kernel_spmd(
        nc, in_maps, core_ids=list(range(N_CORES)), trace=trace, **kw
    )
    out = np.concatenate(
        [np.asarray(r["out"], dtype=np.float32) for r in res.results], axis=0
    )
    return out, res


def kernel(mat_1, mat_2):
    return run(mat_1, mat_2)[0]
